# revision 19
# baseline (speedup 1.0000x reference)
"""Multi-head cross-attention kernel for Trainium2, 8 NeuronCores.

Problem: nn_MultiHeadAttention (H=32 heads, B=8, Lq=Lk=1024, E=128, D=512).

    keys   = einsum('bkd,hde->hbke', states, Wk) + bk
    values = einsum('bkd,hde->hbke', states, Wv) + bv
    attn   = softmax(einsum('bqe,hbke->hbqk', query, keys) / sqrt(E))
    ctx    = einsum('hbqk,hbke->hbqe', attn, values)  -> concat heads
    out    = ctx @ Wo + bo

Sharding: data parallel over batch B=8 -> one batch element per core; no
collectives needed.  Per-core dataflow:

  K^T[h] = Wk[h]-chunks @ states^T        [E, Lk] f32r (copies: ACT/DVE)
  V[h]   = states^T-blocks @ Wv-packed    [Lk-chunk, 4*E] -> SBUF fp16
  S^T    = K^T-block @ query^T            [Lk-chunk, Lq] f32r chunked over Lk
  P      = exp(S^T * 1/sqrt(E))           ACT, fp16 output (scores are O(1),
                                          exp in [0.03, 30] - fp16 safe)
  T      = sum_chunks P                   DVE fp16 running adds (2x mode)
  rowsum = ones[128,128] @ T              ONE matmul per head (vs 8 if the
                                          PE re-streamed every P chunk)
  ctx^T  = V-chunk @ P-chunks             [E, Lq] psum accum, fp16 operands
  ctxn   = ctx^T * recip(rowsum)          DVE
  out^T += Wo[h] @ ctxn                   [E, Lq] SBUF accumulation, DVE

The rowsum restructure is the point: a PE-side rowsum must re-stream all of
P through the array (the PE reads 128 elem/cycle, so ones@P costs as much as
the AV matmul).  Summing the 8 chunk tensors elementwise on the otherwise
idle DVE (fp16 -> 2x rate) leaves the PE only one ones-matmul per head,
cutting PE work by ~21%.

fp16 for P/V adds ~0.05% relative noise on attention weights/values
(simulated end-to-end rel err ~3e-4 vs 2.3e-4 for the all-f32r kernel).
Matmuls stay f32r (full PE rate at N=512) for K/S/projections.  Two exact
bias simplifications: bk dropped (softmax row-shift invariance), bv folded
into the output bias on the host (softmax rows sum to 1).

Emission order software-pipelines the PE: each head's chunk loop carries the
next head's K^T projection, the next group's V projection (2 chunks/head),
and the previous head's output projection as filler between each chunk's S
and AV matmuls, so the PE never waits on the ACT exp chain.
"""

import numpy as np

import concourse.bass as bass
import concourse.mybir as mybir
import concourse.tile as tile
from concourse import bacc
from concourse.bass_utils import run_bass_kernel_spmd

H, E, D = 32, 128, 512
B, LQ, LK = 8, 1024, 1024
NDC = D // 128    # 4 contraction chunks for the projections
NLK = LK // 128   # 8 key chunks
HPG = 4           # heads per group for the packed V computation
NG = H // HPG
SCALE = 1.0 / float(np.sqrt(E))

F32 = mybir.dt.float32
F32R = mybir.dt.float32r
F16 = mybir.dt.float16
EXP = mybir.ActivationFunctionType.Exp
COPY = mybir.ActivationFunctionType.Copy

N_CORES = 8


def _build_kernel(tc, qT, sT, wk, wv, wo, bo2, ones, outT):
    nc = tc.nc
    with (
        tc.tile_pool(name="const", bufs=1) as cpool,
        tc.tile_pool(name="wkp", bufs=2) as wkp,
        tc.tile_pool(name="wvp", bufs=2) as wvp,
        tc.tile_pool(name="wop", bufs=2) as wop,
        tc.tile_pool(name="ktp", bufs=2) as ktp,
        tc.tile_pool(name="vp", bufs=2) as vpool,
        tc.tile_pool(name="pp", bufs=5) as ppool,
        tc.tile_pool(name="tp", bufs=6) as tpool,
        tc.tile_pool(name="tmp", bufs=4) as tmpool,
        tc.tile_pool(name="rcp", bufs=2) as rpool,
        tc.tile_pool(name="cxp", bufs=2) as cxpool,
        tc.tile_pool(name="ps_s", bufs=2, space="PSUM") as ps_s,
        tc.tile_pool(name="ps_c", bufs=1, space="PSUM") as ps_c,
        tc.tile_pool(name="ps_t", bufs=2, space="PSUM") as ps_t,
    ):
        # ---- resident inputs ----
        # st is on the critical path to the first K/V matmuls; q/ones/bo2
        # queue behind it.
        # half-major DMA order: the first K-projection (which only reads
        # columns 0:512 for its half-0 matmuls) can start ~3us earlier
        st_sb = cpool.tile([128, NDC, LK], F32R)
        for half in range(2):
            sl = bass.ts(half, 512)
            for c in range(NDC):
                nc.sync.dma_start(st_sb[:, c, sl], sT[c * 128:(c + 1) * 128, sl])
        q_sb = cpool.tile([E, LQ], F32R)
        ones_sb = cpool.tile([128, 128], F16)
        bo2_sb = cpool.tile([E, 1], F32)
        out_acc = cpool.tile([E, LQ], F32)

        kt_by_head = {}
        v_by_group = {}
        wv_by_group = {}
        p_tiles = {}      # (h, c) -> fp16 P tile
        t_by_head = {}    # h -> fp16 chunk-sum tile
        state = {}        # rotating per-head state (ctxn, psum halves, ...)

        def dma_wk(h):
            wk_sb = wkp.tile([128, NDC, E], F32R, tag="wk", name="wk_sb")
            for c in range(NDC):
                nc.sync.dma_start(wk_sb[:, c, :], wk[h, c * 128:(c + 1) * 128, :])
            state[("wk", h)] = wk_sb

        def dma_wv(g):
            wv_sb = wvp.tile([128, NDC, HPG * E], F32R, tag="wv", name="wv_sb")
            for c in range(NDC):
                nc.sync.dma_start(
                    wv_sb[:, c, :],
                    wv[c * 128:(c + 1) * 128, g * HPG * E:(g + 1) * HPG * E])
            wv_by_group[g] = wv_sb

        def emit_k_half(h, half):
            """Half of the K^T projection for head h (4 matmuls, N=512)."""
            wk_sb = state[("wk", h)]
            if half == 0:
                kt_by_head[h] = ktp.tile([E, LK], F32R, tag="kt", name="kt_sb")
            kt_sb = kt_by_head[h]
            sl = bass.ts(half, 512)
            ps_k = ps_t.tile([E, 512], F32, tag="t", name="ps_k")
            for c in range(NDC):
                nc.tensor.matmul(ps_k[:], (wk_sb[:, c, :]),
                                 (st_sb[:, c, sl]),
                                 start=(c == 0), stop=(c == NDC - 1))
            if half == 0:
                nc.scalar.activation(kt_sb[:, sl], ps_k[:], COPY)
            else:
                nc.vector.tensor_copy(kt_sb[:, sl], ps_k[:])

        def emit_v_chunk(g, lk):
            """One Lk-chunk of the packed V projection for group g (fp16)."""
            if lk == 0:
                v_by_group[g] = vpool.tile([128, NLK, HPG * E], F16,
                                           tag="v", name="v_sb")
            v_sb = v_by_group[g]
            wv_sb = wv_by_group[g]
            ps_v = ps_t.tile([128, HPG * E], F32, tag="t", name="ps_v")
            for c in range(NDC):
                nc.tensor.matmul(
                    ps_v[:], (st_sb[:, c, lk * 128:(lk + 1) * 128]),
                    (wv_sb[:, c, :]), start=(c == 0), stop=(c == NDC - 1))
            # DVE copy: the ACT budget is consumed by the exp chain
            nc.vector.tensor_copy(v_sb[:, lk, :], ps_v[:])

        def emit_s(h, c):
            """S^T chunk c for head h + its exp (fp16 out)."""
            kt_sb = kt_by_head[h]
            pss = ps_s.tile([128, LQ], F32, tag="s", name="ps_s")
            for half in range(2):
                sl = bass.ts(half, 512)
                nc.tensor.matmul(pss[:, sl],
                                 (kt_sb[:, c * 128:(c + 1) * 128]),
                                 (q_sb[:, sl]), start=True, stop=True)
            p_sb = ppool.tile([128, LQ], F16, tag="p", name="p_sb")
            nc.scalar.activation(p_sb[:], pss[:], EXP, scale=SCALE)
            p_tiles[(h, c)] = p_sb

        def emit_av(h, c):
            """AV chunk c accumulating into the two psum halves."""
            hh = h % HPG
            v_sb = v_by_group[h // HPG]
            p_sb = p_tiles[(h, c)]
            for half in range(2):
                sl = bass.ts(half, 512)
                pc = state[("pc", h, half)]
                nc.tensor.matmul(pc[:],
                                 (v_sb[:, c, hh * E:(hh + 1) * E]),
                                 (p_sb[:, sl]),
                                 start=(c == 0), stop=(c == NLK - 1))

        def emit_tadd(h, c):
            """DVE pairwise chunk-sums of P (fp16, 2x mode).

            Four partial sums T01/T23/T45/T67: each add lands right after
            its exp, so the head-boundary critical path is one add after
            the last exp.  Fewer partials would shave PE ones-matmuls but
            pushes ACT+DVE load too close to the PE's: one stall per head
            then drops the chip-wide clock ~20% (measured), which
            saturates ACT and locks the kernel in the slow state."""
            if c % 2 == 0:
                return
            t_sb = tpool.tile([128, LQ], F16, tag="T", name="t_sb")
            nc.vector.tensor_add(t_sb[:], p_tiles.pop((h, c - 1))[:],
                                 p_tiles.pop((h, c))[:])
            t_by_head.setdefault(h, []).append(t_sb)

        def emit_tmerge(h, i):
            """Merge two pair-sums on the (otherwise idle) GpSimd so the
            PE's ones-matmul only streams 2 partials instead of 4."""
            parts = t_by_head[h]
            m_sb = tmpool.tile([128, LQ], F16, tag="Tm", name="m_sb")
            nc.gpsimd.tensor_add(m_sb[:], parts[2 * i][:], parts[2 * i + 1][:])
            state.setdefault(("tm", h), []).append(m_sb)

        def emit_norm(h):
            """rowsum (ones-matmuls over the merged partials) + recip + ctxn."""
            t_by_head.pop(h)
            t_parts = state.pop(("tm", h))
            recip_sb = rpool.tile([128, LQ], F32, tag="recip", name="recip_sb")
            ctxn_sb = cxpool.tile([E, LQ], F32R, tag="ctxn", name="ctxn_sb")
            ps_rs = [ps_t.tile([128, 512], F32, tag="t", name="ps_r")
                     for _ in range(2)]
            for i, t_sb in enumerate(t_parts):
                for half in range(2):
                    sl = bass.ts(half, 512)
                    nc.tensor.matmul(ps_rs[half][:], (ones_sb[:]), (t_sb[:, sl]),
                                     start=(i == 0), stop=(i == len(t_parts) - 1))
            for half in range(2):
                sl = bass.ts(half, 512)
                nc.vector.reciprocal_approx_fast(recip_sb[:, sl], ps_rs[half][:])
            for half in range(2):
                sl = bass.ts(half, 512)
                nc.vector.tensor_mul(ctxn_sb[:, sl],
                                     state.pop(("pc", h, half))[:],
                                     recip_sb[:, sl])
            state[("ctxn", h)] = ctxn_sb

        def emit_proj_half(h, half):
            """Half of the output projection for head h + DVE accumulate."""
            ctxn_sb = state[("ctxn", h)]
            wo_sb = state[("wo", h)]
            sl = bass.ts(half, 512)
            ps_p = ps_t.tile([E, 512], F32, tag="t", name="ps_p")
            nc.tensor.matmul(ps_p[:], (wo_sb[:]), (ctxn_sb[:, sl]),
                             start=True, stop=True)
            if h == 0:
                nc.vector.tensor_scalar_add(out_acc[:, sl], ps_p[:],
                                            bo2_sb[:, 0:1])
            else:
                nc.vector.tensor_add(out_acc[:, sl], out_acc[:, sl], ps_p[:])
            if half == 1:
                state.pop(("ctxn", h))
                state.pop(("wo", h))

        # ================= prologue =================
        dma_wk(0)
        emit_k_half(0, 0)
        nc.sync.dma_start(q_sb[:, bass.ts(0, 512)], qT[:, bass.ts(0, 512)])
        emit_k_half(0, 1)
        nc.sync.dma_start(q_sb[:, bass.ts(1, 512)], qT[:, bass.ts(1, 512)])
        dma_wv(0)
        nc.sync.dma_start(ones_sb[:], ones[:])
        nc.sync.dma_start(bo2_sb[:], bo2[:])
        emit_s(0, 0)
        for lk in range(NLK):
            emit_v_chunk(0, lk)
        emit_s(0, 1)
        dma_wk(1)
        dma_wv(1)
        emit_k_half(1, 0)
        emit_k_half(1, 1)

        # ================= head loop =================
        for h in range(H):
            hh = h % HPG
            g = h // HPG
            # boundary B(h) for h >= 1 (h == 0 handled by the prologue).
            # The K projection of head h+1 runs BEFORE the previous head's
            # normalization: it covers the exp7 -> T-add -> GpSimd-merge
            # latency so the ones-matmul input is ready when the PE gets
            # there.
            if h >= 1:
                if h + 1 < H:
                    dma_wk(h + 1)
                if hh == 0 and g + 1 < NG:
                    dma_wv(g + 1)
                wo_sb = wop.tile([E, E], F32R, tag="wo", name="wo_sb")
                nc.sync.dma_start(wo_sb[:], wo[(h - 1) * E:h * E, :])
                state[("wo", h - 1)] = wo_sb
                emit_s(h, 0)
                if h + 1 < H:
                    emit_k_half(h + 1, 0)
                emit_s(h, 1)
                if h + 1 < H:
                    emit_k_half(h + 1, 1)
                emit_norm(h - 1)          # ones-matmuls + recip + muls

            state[("pc", h, 0)] = ps_c.tile([E, 512], F32, tag="cA", name="ps_cA")
            state[("pc", h, 1)] = ps_c.tile([E, 512], F32, tag="cB", name="ps_cB")

            for c in range(NLK):
                # PE fillers between this chunk's dependencies; the
                # projection sits at c2/c3 (its ctxn comes from DVE muls
                # that trail the boundary ones-matmuls by ~1.5us)
                if c == 2 and h >= 1:
                    emit_proj_half(h - 1, 0)
                if c == 3 and h >= 1:
                    emit_proj_half(h - 1, 1)
                if c in (4, 5) and g + 1 < NG:
                    emit_v_chunk(g + 1, 2 * hh + (c - 4))
                emit_av(h, c)
                emit_tadd(h, c)
                if c == 3:
                    emit_tmerge(h, 0)
                if c == 7:
                    emit_tmerge(h, 1)
                if c + 2 < NLK:
                    emit_s(h, c + 2)

        # ================= tail =================
        wo_sb = wop.tile([E, E], F32R, tag="wo", name="wo_sb")
        nc.sync.dma_start(wo_sb[:], wo[(H - 1) * E:H * E, :])
        state[("wo", H - 1)] = wo_sb
        emit_norm(H - 1)
        for half in range(2):
            emit_proj_half(H - 1, half)
            nc.sync.dma_start(outT[:, bass.ts(half, 512)],
                              out_acc[:, bass.ts(half, 512)])


def build_program():
    nc = bacc.Bacc("TRN2", target_bir_lowering=False, debug=False,
                   num_devices=N_CORES)
    qT = nc.dram_tensor("qT", [E, LQ], F32R, kind="ExternalInput").ap()
    sT = nc.dram_tensor("sT", [D, LK], F32R, kind="ExternalInput").ap()
    wk = nc.dram_tensor("wk", [H, D, E], F32R, kind="ExternalInput").ap()
    wv = nc.dram_tensor("wv", [D, H * E], F32R, kind="ExternalInput").ap()
    wo = nc.dram_tensor("wo", [H * E, E], F32R, kind="ExternalInput").ap()
    bo2 = nc.dram_tensor("bo2", [E, 1], F32, kind="ExternalInput").ap()
    ones = nc.dram_tensor("ones", [128, 128], F16, kind="ExternalInput").ap()
    outT = nc.dram_tensor("outT", [E, LQ], F32, kind="ExternalOutput").ap()

    with tile.TileContext(nc) as tc:
        _build_kernel(tc, qT, sT, wk, wv, wo, bo2, ones, outT)
    nc.compile()
    return nc


def _round_f32r(a):
    """Round fp32 -> fp32r (11-bit mantissa, low 12 bits zero), RN-even.

    The PE's fp32r datapath keeps sign+8exp+11mantissa; the BIR verifier
    requires fp32r matmul operands to be pre-rounded, and rounding on the
    host gives round-to-nearest instead of hardware truncation.
    """
    b = np.ascontiguousarray(a, dtype=np.float32).view(np.uint32)
    b = b + 0x7FF + ((b >> 12) & 1)
    b &= np.uint32(0xFFFFF000)
    return b.view(np.float32)


def make_in_maps(query, states, Wk, bk, Wv, bv, Wo, bo):
    """Shard the full inputs into per-core input maps (host-side prep)."""
    wv_packed = np.ascontiguousarray(
        np.transpose(Wv, (1, 0, 2)).reshape(D, H * E))
    # fold bv through the output projection: softmax rows sum to 1
    bo2 = bo.astype(np.float64).copy()
    for h in range(H):
        bo2 += bv[h].astype(np.float64) @ Wo[h * E:(h + 1) * E].astype(np.float64)
    bo2 = bo2.astype(np.float32).reshape(E, 1)
    wk_c = _round_f32r(Wk)
    wo_c = _round_f32r(Wo)
    wv_packed = _round_f32r(wv_packed)

    in_maps = []
    for b in range(B):
        in_maps.append({
            "qT": _round_f32r(query[b].T),
            "sT": _round_f32r(states[b].T),
            "wk": wk_c,
            "wv": wv_packed,
            "wo": wo_c,
            "bo2": bo2,
            "ones": np.ones((128, 128), dtype=np.float16),
        })
    return in_maps


_PROGRAM_CACHE = {}


def _get_program():
    if "nc" not in _PROGRAM_CACHE:
        _PROGRAM_CACHE["nc"] = build_program()
    return _PROGRAM_CACHE["nc"]


def kernel(query, states, Wk, bk, Wv, bv, Wo, bo, _trace=False, _tmpdir=None):
    args = [np.asarray(a, dtype=np.float32)
            for a in (query, states, Wk, bk, Wv, bv, Wo, bo)]
    nc = _get_program()
    in_maps = make_in_maps(*args)
    last_err = None
    for _attempt in range(2):  # one retry for transient device errors
        try:
            res = run_bass_kernel_spmd(nc, in_maps,
                                       core_ids=list(range(N_CORES)),
                                       trace=_trace, tmpdir=_tmpdir)
            break
        except Exception as e:  # noqa: BLE001
            last_err = e
    else:
        raise last_err
    out = np.stack([res.results[b]["outT"].T for b in range(B)])
    out = np.ascontiguousarray(out.astype(np.float32))
    if _trace:
        kernel.last_exec_time_ns = res.exec_time_ns
        kernel.last_results = res
    return out


if __name__ == "__main__":
    rng = np.random.default_rng(0)
    inputs = {
        "query": rng.standard_normal((B, LQ, E), dtype=np.float32),
        "states": rng.standard_normal((B, LK, D), dtype=np.float32),
        "Wk": rng.uniform(-0.04, 0.04, (H, D, E)).astype(np.float32),
        "bk": rng.uniform(-0.04, 0.04, (H, E)).astype(np.float32),
        "Wv": rng.uniform(-0.04, 0.04, (H, D, E)).astype(np.float32),
        "bv": rng.uniform(-0.04, 0.04, (H, E)).astype(np.float32),
        "Wo": rng.uniform(-0.015, 0.015, (H * E, E)).astype(np.float32),
        "bo": rng.uniform(-0.015, 0.015, (E,)).astype(np.float32),
    }
    out = kernel(**inputs)
    print(out.shape, out.dtype)


# revision 20
# speedup vs baseline: 1.0437x; 1.0437x over previous
"""Multi-head cross-attention kernel for Trainium2, 8 NeuronCores.

Problem: nn_MultiHeadAttention (H=32 heads, B=8, Lq=Lk=1024, E=128, D=512).

    keys   = einsum('bkd,hde->hbke', states, Wk) + bk
    values = einsum('bkd,hde->hbke', states, Wv) + bv
    attn   = softmax(einsum('bqe,hbke->hbqk', query, keys) / sqrt(E))
    ctx    = einsum('hbqk,hbke->hbqe', attn, values)  -> concat heads
    out    = ctx @ Wo + bo

Sharding: data parallel over batch B=8 -> one batch element per core; no
collectives needed.  Per-core dataflow:

  K^T[h] = Wk[h]-chunks @ states^T        [E, Lk] f32r (copies: ACT/DVE)
  V[h]   = states^T-blocks @ Wv-packed    [Lk-chunk, 4*E] -> SBUF fp16
  S^T    = K^T-block @ query^T            [Lk-chunk, Lq] f32r chunked over Lk
  P      = exp(S^T * 1/sqrt(E))           ACT, fp16 output (scores are O(1),
                                          exp in [0.03, 30] - fp16 safe)
  T      = sum_chunks P                   DVE fp16 running adds (2x mode)
  rowsum = ones[128,128] @ T              ONE matmul per head (vs 8 if the
                                          PE re-streamed every P chunk)
  ctx^T  = V-chunk @ P-chunks             [E, Lq] psum accum, fp16 operands
  ctxn   = ctx^T * recip(rowsum)          DVE
  out^T += Wo[h] @ ctxn                   [E, Lq] SBUF accumulation, DVE

The rowsum restructure is the point: a PE-side rowsum must re-stream all of
P through the array (the PE reads 128 elem/cycle, so ones@P costs as much as
the AV matmul).  Summing the 8 chunk tensors elementwise on the otherwise
idle DVE (fp16 -> 2x rate) leaves the PE only one ones-matmul per head,
cutting PE work by ~21%.

fp16 for P/V adds ~0.05% relative noise on attention weights/values
(simulated end-to-end rel err ~3e-4 vs 2.3e-4 for the all-f32r kernel).
Matmuls stay f32r (full PE rate at N=512) for K/S/projections.  Two exact
bias simplifications: bk dropped (softmax row-shift invariance), bv folded
into the output bias on the host (softmax rows sum to 1).

Emission order software-pipelines the PE: each head's chunk loop carries the
next head's K^T projection, the next group's V projection (2 chunks/head),
and the previous head's output projection as filler between each chunk's S
and AV matmuls, so the PE never waits on the ACT exp chain.
"""

import numpy as np

import concourse.bass as bass
import concourse.mybir as mybir
import concourse.tile as tile
from concourse import bacc
from concourse.bass_utils import run_bass_kernel_spmd

H, E, D = 32, 128, 512
B, LQ, LK = 8, 1024, 1024
NDC = D // 128    # 4 contraction chunks for the projections
NLK = LK // 128   # 8 key chunks
HPG = 4           # heads per group for the packed V computation
NG = H // HPG
SCALE = 1.0 / float(np.sqrt(E))

F32 = mybir.dt.float32
F32R = mybir.dt.float32r
F16 = mybir.dt.float16
EXP = mybir.ActivationFunctionType.Exp
COPY = mybir.ActivationFunctionType.Copy

N_CORES = 8


def _build_kernel(tc, qT, sT, wk, wv, wo, bo2, ones, outT):
    nc = tc.nc
    with (
        tc.tile_pool(name="const", bufs=1) as cpool,
        tc.tile_pool(name="wkp", bufs=2) as wkp,
        tc.tile_pool(name="wvp", bufs=2) as wvp,
        tc.tile_pool(name="wop", bufs=2) as wop,
        tc.tile_pool(name="ktp", bufs=2) as ktp,
        tc.tile_pool(name="vp", bufs=2) as vpool,
        tc.tile_pool(name="pp", bufs=5) as ppool,
        tc.tile_pool(name="tp", bufs=6) as tpool,
        tc.tile_pool(name="tmp", bufs=4) as tmpool,
        tc.tile_pool(name="rcp", bufs=2) as rpool,
        tc.tile_pool(name="cxp", bufs=2) as cxpool,
        tc.tile_pool(name="ps_s", bufs=2, space="PSUM") as ps_s,
        tc.tile_pool(name="ps_c", bufs=1, space="PSUM") as ps_c,
        tc.tile_pool(name="ps_t", bufs=2, space="PSUM") as ps_t,
    ):
        # ---- resident inputs ----
        # st is on the critical path to the first K/V matmuls; q/ones/bo2
        # queue behind it.
        # half-major DMA order: the first K-projection (which only reads
        # columns 0:512 for its half-0 matmuls) can start ~3us earlier
        st_sb = cpool.tile([128, NDC, LK], F32R)
        for half in range(2):
            sl = bass.ts(half, 512)
            for c in range(NDC):
                nc.sync.dma_start(st_sb[:, c, sl], sT[c * 128:(c + 1) * 128, sl])
        q_sb = cpool.tile([E, LQ], F32R)
        ones_sb = cpool.tile([128, 128], F16)
        bo2_sb = cpool.tile([E, 1], F32)
        out_acc = cpool.tile([E, LQ], F32)

        kt_by_head = {}
        v_by_group = {}
        wv_by_group = {}
        p_tiles = {}      # (h, c) -> fp16 P tile
        t_by_head = {}    # h -> fp16 chunk-sum tile
        state = {}        # rotating per-head state (ctxn, psum halves, ...)

        def dma_wk(h):
            wk_sb = wkp.tile([128, NDC, E], F32R, tag="wk", name="wk_sb")
            for c in range(NDC):
                nc.sync.dma_start(wk_sb[:, c, :], wk[h, c * 128:(c + 1) * 128, :])
            state[("wk", h)] = wk_sb

        def dma_wv(g):
            wv_sb = wvp.tile([128, NDC, HPG * E], F32R, tag="wv", name="wv_sb")
            for c in range(NDC):
                nc.sync.dma_start(
                    wv_sb[:, c, :],
                    wv[c * 128:(c + 1) * 128, g * HPG * E:(g + 1) * HPG * E])
            wv_by_group[g] = wv_sb

        def emit_k_half(h, half):
            """Half of the K^T projection for head h (4 matmuls, N=512)."""
            wk_sb = state[("wk", h)]
            if half == 0:
                kt_by_head[h] = ktp.tile([E, LK], F32R, tag="kt", name="kt_sb")
            kt_sb = kt_by_head[h]
            sl = bass.ts(half, 512)
            ps_k = ps_t.tile([E, 512], F32, tag="t", name="ps_k")
            for c in range(NDC):
                nc.tensor.matmul(ps_k[:], (wk_sb[:, c, :]),
                                 (st_sb[:, c, sl]),
                                 start=(c == 0), stop=(c == NDC - 1))
            if half == 0:
                nc.scalar.activation(kt_sb[:, sl], ps_k[:], COPY)
            else:
                nc.vector.tensor_copy(kt_sb[:, sl], ps_k[:])

        def emit_v_chunk(g, lk):
            """One Lk-chunk of the packed V projection for group g (fp16)."""
            if lk == 0:
                v_by_group[g] = vpool.tile([128, NLK, HPG * E], F16,
                                           tag="v", name="v_sb")
            v_sb = v_by_group[g]
            wv_sb = wv_by_group[g]
            ps_v = ps_t.tile([128, HPG * E], F32, tag="t", name="ps_v")
            for c in range(NDC):
                nc.tensor.matmul(
                    ps_v[:], (st_sb[:, c, lk * 128:(lk + 1) * 128]),
                    (wv_sb[:, c, :]), start=(c == 0), stop=(c == NDC - 1))
            # DVE copy: the ACT budget is consumed by the exp chain
            nc.vector.tensor_copy(v_sb[:, lk, :], ps_v[:])

        def emit_s(h, c):
            """S^T chunk c for head h + its exp (fp16 out)."""
            kt_sb = kt_by_head[h]
            pss = ps_s.tile([128, LQ], F32, tag="s", name="ps_s")
            for half in range(2):
                sl = bass.ts(half, 512)
                nc.tensor.matmul(pss[:, sl],
                                 (kt_sb[:, c * 128:(c + 1) * 128]),
                                 (q_sb[:, sl]), start=True, stop=True)
            p_sb = ppool.tile([128, LQ], F16, tag="p", name="p_sb")
            nc.scalar.activation(p_sb[:], pss[:], EXP, scale=SCALE)
            p_tiles[(h, c)] = p_sb

        def emit_av(h, c):
            """AV chunk c accumulating into the two psum halves."""
            hh = h % HPG
            v_sb = v_by_group[h // HPG]
            p_sb = p_tiles[(h, c)]
            for half in range(2):
                sl = bass.ts(half, 512)
                pc = state[("pc", h, half)]
                nc.tensor.matmul(pc[:],
                                 (v_sb[:, c, hh * E:(hh + 1) * E]),
                                 (p_sb[:, sl]),
                                 start=(c == 0), stop=(c == NLK - 1))

        def emit_tadd(h, c):
            """DVE pairwise chunk-sums of P (fp16, 2x mode).

            Four partial sums T01/T23/T45/T67: each add lands right after
            its exp, so the head-boundary critical path is one add after
            the last exp.  Fewer partials would shave PE ones-matmuls but
            pushes ACT+DVE load too close to the PE's: one stall per head
            then drops the chip-wide clock ~20% (measured), which
            saturates ACT and locks the kernel in the slow state."""
            if c % 2 == 0:
                return
            t_sb = tpool.tile([128, LQ], F16, tag="T", name="t_sb")
            nc.vector.tensor_add(t_sb[:], p_tiles.pop((h, c - 1))[:],
                                 p_tiles.pop((h, c))[:])
            t_by_head.setdefault(h, []).append(t_sb)

        def emit_tmerge(h, i):
            """Merge two pair-sums so the PE's ones-matmul only streams 2
            partials instead of 4.  Merge 0 (mid-loop, off the critical
            path) goes to the otherwise-idle GpSimd (~2.1us); merge 1 sits
            on the head-boundary path right after the last exp, so it runs
            on the DVE (~0.7us) where the boundary PE filler covers it."""
            parts = t_by_head[h]
            m_sb = tmpool.tile([128, LQ], F16, tag="Tm", name="m_sb")
            eng = nc.gpsimd if i == 0 else nc.vector
            eng.tensor_add(m_sb[:], parts[2 * i][:], parts[2 * i + 1][:])
            state.setdefault(("tm", h), []).append(m_sb)

        def emit_norm(h):
            """rowsum (ones-matmuls over the merged partials) + recip + ctxn."""
            t_by_head.pop(h)
            t_parts = state.pop(("tm", h))
            recip_sb = rpool.tile([128, LQ], F32, tag="recip", name="recip_sb")
            ctxn_sb = cxpool.tile([E, LQ], F32R, tag="ctxn", name="ctxn_sb")
            ps_rs = [ps_t.tile([128, 512], F32, tag="t", name="ps_r")
                     for _ in range(2)]
            for i, t_sb in enumerate(t_parts):
                for half in range(2):
                    sl = bass.ts(half, 512)
                    nc.tensor.matmul(ps_rs[half][:], (ones_sb[:]), (t_sb[:, sl]),
                                     start=(i == 0), stop=(i == len(t_parts) - 1))
            for half in range(2):
                sl = bass.ts(half, 512)
                nc.vector.reciprocal_approx_fast(recip_sb[:, sl], ps_rs[half][:])
            for half in range(2):
                sl = bass.ts(half, 512)
                nc.vector.tensor_mul(ctxn_sb[:, sl],
                                     state.pop(("pc", h, half))[:],
                                     recip_sb[:, sl])
            state[("ctxn", h)] = ctxn_sb

        def emit_proj_half(h, half):
            """Half of the output projection for head h + DVE accumulate."""
            ctxn_sb = state[("ctxn", h)]
            wo_sb = state[("wo", h)]
            sl = bass.ts(half, 512)
            ps_p = ps_t.tile([E, 512], F32, tag="t", name="ps_p")
            nc.tensor.matmul(ps_p[:], (wo_sb[:]), (ctxn_sb[:, sl]),
                             start=True, stop=True)
            if h == 0:
                nc.vector.tensor_scalar_add(out_acc[:, sl], ps_p[:],
                                            bo2_sb[:, 0:1])
            else:
                nc.vector.tensor_add(out_acc[:, sl], out_acc[:, sl], ps_p[:])
            if half == 1:
                state.pop(("ctxn", h))
                state.pop(("wo", h))

        # ================= prologue =================
        dma_wk(0)
        emit_k_half(0, 0)
        nc.sync.dma_start(q_sb[:, bass.ts(0, 512)], qT[:, bass.ts(0, 512)])
        emit_k_half(0, 1)
        nc.sync.dma_start(q_sb[:, bass.ts(1, 512)], qT[:, bass.ts(1, 512)])
        dma_wv(0)
        nc.sync.dma_start(ones_sb[:], ones[:])
        nc.sync.dma_start(bo2_sb[:], bo2[:])
        emit_s(0, 0)
        for lk in range(NLK):
            emit_v_chunk(0, lk)
        emit_s(0, 1)
        dma_wk(1)
        dma_wv(1)
        emit_k_half(1, 0)
        emit_k_half(1, 1)

        # ================= head loop =================
        for h in range(H):
            hh = h % HPG
            g = h // HPG
            # boundary B(h) for h >= 1 (h == 0 handled by the prologue).
            # The K projection of head h+1 runs BEFORE the previous head's
            # normalization: it covers the exp7 -> T-add -> GpSimd-merge
            # latency so the ones-matmul input is ready when the PE gets
            # there.
            if h >= 1:
                if h + 1 < H:
                    dma_wk(h + 1)
                if hh == 0 and g + 1 < NG:
                    dma_wv(g + 1)
                wo_sb = wop.tile([E, E], F32R, tag="wo", name="wo_sb")
                nc.sync.dma_start(wo_sb[:], wo[(h - 1) * E:h * E, :])
                state[("wo", h - 1)] = wo_sb
                emit_s(h, 0)
                if h + 1 < H:
                    emit_k_half(h + 1, 0)
                emit_s(h, 1)
                if h + 1 < H:
                    emit_k_half(h + 1, 1)
                emit_norm(h - 1)          # ones-matmuls + recip + muls

            state[("pc", h, 0)] = ps_c.tile([E, 512], F32, tag="cA", name="ps_cA")
            state[("pc", h, 1)] = ps_c.tile([E, 512], F32, tag="cB", name="ps_cB")

            for c in range(NLK):
                # PE fillers between this chunk's dependencies; the
                # projection sits at c2/c3 (its ctxn comes from DVE muls
                # that trail the boundary ones-matmuls by ~1.5us)
                if c == 2 and h >= 1:
                    emit_proj_half(h - 1, 0)
                if c == 3 and h >= 1:
                    emit_proj_half(h - 1, 1)
                if c in (4, 5) and g + 1 < NG:
                    emit_v_chunk(g + 1, 2 * hh + (c - 4))
                emit_av(h, c)
                emit_tadd(h, c)
                if c == 3:
                    emit_tmerge(h, 0)
                if c == 7:
                    emit_tmerge(h, 1)
                if c + 2 < NLK:
                    emit_s(h, c + 2)

        # ================= tail =================
        wo_sb = wop.tile([E, E], F32R, tag="wo", name="wo_sb")
        nc.sync.dma_start(wo_sb[:], wo[(H - 1) * E:H * E, :])
        state[("wo", H - 1)] = wo_sb
        emit_norm(H - 1)
        for half in range(2):
            emit_proj_half(H - 1, half)
            nc.sync.dma_start(outT[:, bass.ts(half, 512)],
                              out_acc[:, bass.ts(half, 512)])


def build_program():
    nc = bacc.Bacc("TRN2", target_bir_lowering=False, debug=False,
                   num_devices=N_CORES)
    qT = nc.dram_tensor("qT", [E, LQ], F32R, kind="ExternalInput").ap()
    sT = nc.dram_tensor("sT", [D, LK], F32R, kind="ExternalInput").ap()
    wk = nc.dram_tensor("wk", [H, D, E], F32R, kind="ExternalInput").ap()
    wv = nc.dram_tensor("wv", [D, H * E], F32R, kind="ExternalInput").ap()
    wo = nc.dram_tensor("wo", [H * E, E], F32R, kind="ExternalInput").ap()
    bo2 = nc.dram_tensor("bo2", [E, 1], F32, kind="ExternalInput").ap()
    ones = nc.dram_tensor("ones", [128, 128], F16, kind="ExternalInput").ap()
    outT = nc.dram_tensor("outT", [E, LQ], F32, kind="ExternalOutput").ap()

    with tile.TileContext(nc) as tc:
        _build_kernel(tc, qT, sT, wk, wv, wo, bo2, ones, outT)
    nc.compile()
    return nc


def _round_f32r(a):
    """Round fp32 -> fp32r (11-bit mantissa, low 12 bits zero), RN-even.

    The PE's fp32r datapath keeps sign+8exp+11mantissa; the BIR verifier
    requires fp32r matmul operands to be pre-rounded, and rounding on the
    host gives round-to-nearest instead of hardware truncation.
    """
    b = np.ascontiguousarray(a, dtype=np.float32).view(np.uint32)
    b = b + 0x7FF + ((b >> 12) & 1)
    b &= np.uint32(0xFFFFF000)
    return b.view(np.float32)


def make_in_maps(query, states, Wk, bk, Wv, bv, Wo, bo):
    """Shard the full inputs into per-core input maps (host-side prep)."""
    wv_packed = np.ascontiguousarray(
        np.transpose(Wv, (1, 0, 2)).reshape(D, H * E))
    # fold bv through the output projection: softmax rows sum to 1
    bo2 = bo.astype(np.float64).copy()
    for h in range(H):
        bo2 += bv[h].astype(np.float64) @ Wo[h * E:(h + 1) * E].astype(np.float64)
    bo2 = bo2.astype(np.float32).reshape(E, 1)
    wk_c = _round_f32r(Wk)
    wo_c = _round_f32r(Wo)
    wv_packed = _round_f32r(wv_packed)

    in_maps = []
    for b in range(B):
        in_maps.append({
            "qT": _round_f32r(query[b].T),
            "sT": _round_f32r(states[b].T),
            "wk": wk_c,
            "wv": wv_packed,
            "wo": wo_c,
            "bo2": bo2,
            "ones": np.ones((128, 128), dtype=np.float16),
        })
    return in_maps


_PROGRAM_CACHE = {}


def _get_program():
    if "nc" not in _PROGRAM_CACHE:
        _PROGRAM_CACHE["nc"] = build_program()
    return _PROGRAM_CACHE["nc"]


def kernel(query, states, Wk, bk, Wv, bv, Wo, bo, _trace=False, _tmpdir=None):
    args = [np.asarray(a, dtype=np.float32)
            for a in (query, states, Wk, bk, Wv, bv, Wo, bo)]
    nc = _get_program()
    in_maps = make_in_maps(*args)
    last_err = None
    for _attempt in range(2):  # one retry for transient device errors
        try:
            res = run_bass_kernel_spmd(nc, in_maps,
                                       core_ids=list(range(N_CORES)),
                                       trace=_trace, tmpdir=_tmpdir)
            break
        except Exception as e:  # noqa: BLE001
            last_err = e
    else:
        raise last_err
    out = np.stack([res.results[b]["outT"].T for b in range(B)])
    out = np.ascontiguousarray(out.astype(np.float32))
    if _trace:
        kernel.last_exec_time_ns = res.exec_time_ns
        kernel.last_results = res
    return out


if __name__ == "__main__":
    rng = np.random.default_rng(0)
    inputs = {
        "query": rng.standard_normal((B, LQ, E), dtype=np.float32),
        "states": rng.standard_normal((B, LK, D), dtype=np.float32),
        "Wk": rng.uniform(-0.04, 0.04, (H, D, E)).astype(np.float32),
        "bk": rng.uniform(-0.04, 0.04, (H, E)).astype(np.float32),
        "Wv": rng.uniform(-0.04, 0.04, (H, D, E)).astype(np.float32),
        "bv": rng.uniform(-0.04, 0.04, (H, E)).astype(np.float32),
        "Wo": rng.uniform(-0.015, 0.015, (H * E, E)).astype(np.float32),
        "bo": rng.uniform(-0.015, 0.015, (E,)).astype(np.float32),
    }
    out = kernel(**inputs)
    print(out.shape, out.dtype)


# revision 21
# speedup vs baseline: 1.0441x; 1.0004x over previous
"""Multi-head cross-attention kernel for Trainium2, 8 NeuronCores.

Problem: nn_MultiHeadAttention (H=32 heads, B=8, Lq=Lk=1024, E=128, D=512).

    keys   = einsum('bkd,hde->hbke', states, Wk) + bk
    values = einsum('bkd,hde->hbke', states, Wv) + bv
    attn   = softmax(einsum('bqe,hbke->hbqk', query, keys) / sqrt(E))
    ctx    = einsum('hbqk,hbke->hbqe', attn, values)  -> concat heads
    out    = ctx @ Wo + bo

Sharding: data parallel over batch B=8 -> one batch element per core; no
collectives needed.  Per-core dataflow:

  K^T[h] = Wk[h]-chunks @ states^T        [E, Lk] f32r (copies: ACT/DVE)
  V[h]   = states^T-blocks @ Wv-packed    [Lk-chunk, 4*E] -> SBUF fp16
  S^T    = K^T-block @ query^T            [Lk-chunk, Lq] f32r chunked over Lk
  P      = exp(S^T * 1/sqrt(E))           ACT, fp16 output (scores are O(1),
                                          exp in [0.03, 30] - fp16 safe)
  T      = sum_chunks P                   DVE fp16 running adds (2x mode)
  rowsum = ones[128,128] @ T              ONE matmul per head (vs 8 if the
                                          PE re-streamed every P chunk)
  ctx^T  = V-chunk @ P-chunks             [E, Lq] psum accum, fp16 operands
  ctxn   = ctx^T * recip(rowsum)          DVE
  out^T += Wo[h] @ ctxn                   [E, Lq] SBUF accumulation, DVE

The rowsum restructure is the point: a PE-side rowsum must re-stream all of
P through the array (the PE reads 128 elem/cycle, so ones@P costs as much as
the AV matmul).  Summing the 8 chunk tensors elementwise on the otherwise
idle DVE (fp16 -> 2x rate) leaves the PE only one ones-matmul per head,
cutting PE work by ~21%.

fp16 for P/V adds ~0.05% relative noise on attention weights/values
(simulated end-to-end rel err ~3e-4 vs 2.3e-4 for the all-f32r kernel).
Matmuls stay f32r (full PE rate at N=512) for K/S/projections.  Two exact
bias simplifications: bk dropped (softmax row-shift invariance), bv folded
into the output bias on the host (softmax rows sum to 1).

Emission order software-pipelines the PE: each head's chunk loop carries the
next head's K^T projection, the next group's V projection (2 chunks/head),
and the previous head's output projection as filler between each chunk's S
and AV matmuls, so the PE never waits on the ACT exp chain.
"""

import numpy as np

import concourse.bass as bass
import concourse.mybir as mybir
import concourse.tile as tile
from concourse import bacc
from concourse.bass_utils import run_bass_kernel_spmd

H, E, D = 32, 128, 512
B, LQ, LK = 8, 1024, 1024
NDC = D // 128    # 4 contraction chunks for the projections
NLK = LK // 128   # 8 key chunks
HPG = 4           # heads per group for the packed V computation
NG = H // HPG
SCALE = 1.0 / float(np.sqrt(E))

F32 = mybir.dt.float32
F32R = mybir.dt.float32r
F16 = mybir.dt.float16
EXP = mybir.ActivationFunctionType.Exp
COPY = mybir.ActivationFunctionType.Copy

N_CORES = 8


def _build_kernel(tc, qT, sT, wk, wv, wo, bo2, ones, outT):
    nc = tc.nc
    with (
        tc.tile_pool(name="const", bufs=1) as cpool,
        tc.tile_pool(name="wkp", bufs=2) as wkp,
        tc.tile_pool(name="wvp", bufs=2) as wvp,
        tc.tile_pool(name="wop", bufs=2) as wop,
        tc.tile_pool(name="ktp", bufs=2) as ktp,
        tc.tile_pool(name="vp", bufs=2) as vpool,
        tc.tile_pool(name="pp", bufs=5) as ppool,
        tc.tile_pool(name="tp", bufs=6) as tpool,
        tc.tile_pool(name="tmp", bufs=4) as tmpool,
        tc.tile_pool(name="rcp", bufs=2) as rpool,
        tc.tile_pool(name="cxp", bufs=2) as cxpool,
        tc.tile_pool(name="ps_s", bufs=2, space="PSUM") as ps_s,
        tc.tile_pool(name="ps_c", bufs=1, space="PSUM") as ps_c,
        tc.tile_pool(name="ps_t", bufs=2, space="PSUM") as ps_t,
    ):
        # ---- resident inputs ----
        # st is on the critical path to the first K/V matmuls; q/ones/bo2
        # queue behind it.
        # half-major DMA order: the first K-projection (which only reads
        # columns 0:512 for its half-0 matmuls) can start ~3us earlier
        st_sb = cpool.tile([128, NDC, LK], F32R)
        for half in range(2):
            sl = bass.ts(half, 512)
            for c in range(NDC):
                nc.sync.dma_start(st_sb[:, c, sl], sT[c * 128:(c + 1) * 128, sl])
        q_sb = cpool.tile([E, LQ], F32R)
        ones_sb = cpool.tile([128, 128], F16)
        bo2_sb = cpool.tile([E, 1], F32)
        out_acc = cpool.tile([E, LQ], F32)

        kt_by_head = {}
        v_by_group = {}
        wv_by_group = {}
        p_tiles = {}      # (h, c) -> fp16 P tile
        t_by_head = {}    # h -> fp16 chunk-sum tile
        state = {}        # rotating per-head state (ctxn, psum halves, ...)

        def dma_wk(h):
            wk_sb = wkp.tile([128, NDC, E], F32R, tag="wk", name="wk_sb")
            for c in range(NDC):
                nc.sync.dma_start(wk_sb[:, c, :], wk[h, c * 128:(c + 1) * 128, :])
            state[("wk", h)] = wk_sb

        def dma_wv(g):
            wv_sb = wvp.tile([128, NDC, HPG * E], F32R, tag="wv", name="wv_sb")
            for c in range(NDC):
                nc.sync.dma_start(
                    wv_sb[:, c, :],
                    wv[c * 128:(c + 1) * 128, g * HPG * E:(g + 1) * HPG * E])
            wv_by_group[g] = wv_sb

        def emit_k_half(h, half):
            """Half of the K^T projection for head h (4 matmuls, N=512)."""
            wk_sb = state[("wk", h)]
            if half == 0:
                kt_by_head[h] = ktp.tile([E, LK], F32R, tag="kt", name="kt_sb")
            kt_sb = kt_by_head[h]
            sl = bass.ts(half, 512)
            ps_k = ps_t.tile([E, 512], F32, tag="t", name="ps_k")
            for c in range(NDC):
                nc.tensor.matmul(ps_k[:], (wk_sb[:, c, :]),
                                 (st_sb[:, c, sl]),
                                 start=(c == 0), stop=(c == NDC - 1))
            if half == 0:
                nc.scalar.activation(kt_sb[:, sl], ps_k[:], COPY)
            else:
                nc.vector.tensor_copy(kt_sb[:, sl], ps_k[:])

        def emit_v_chunk(g, lk):
            """One Lk-chunk of the packed V projection for group g (fp16)."""
            if lk == 0:
                v_by_group[g] = vpool.tile([128, NLK, HPG * E], F16,
                                           tag="v", name="v_sb")
            v_sb = v_by_group[g]
            wv_sb = wv_by_group[g]
            ps_v = ps_t.tile([128, HPG * E], F32, tag="t", name="ps_v")
            for c in range(NDC):
                nc.tensor.matmul(
                    ps_v[:], (st_sb[:, c, lk * 128:(lk + 1) * 128]),
                    (wv_sb[:, c, :]), start=(c == 0), stop=(c == NDC - 1))
            # DVE copy: the ACT budget is consumed by the exp chain
            nc.vector.tensor_copy(v_sb[:, lk, :], ps_v[:])

        def emit_s(h, c):
            """S^T chunk c for head h + its exp (fp16 out)."""
            kt_sb = kt_by_head[h]
            pss = ps_s.tile([128, LQ], F32, tag="s", name="ps_s")
            for half in range(2):
                sl = bass.ts(half, 512)
                nc.tensor.matmul(pss[:, sl],
                                 (kt_sb[:, c * 128:(c + 1) * 128]),
                                 (q_sb[:, sl]), start=True, stop=True)
            p_sb = ppool.tile([128, LQ], F16, tag="p", name="p_sb")
            nc.scalar.activation(p_sb[:], pss[:], EXP, scale=SCALE)
            p_tiles[(h, c)] = p_sb

        def emit_av(h, c):
            """AV chunk c accumulating into the two psum halves."""
            hh = h % HPG
            v_sb = v_by_group[h // HPG]
            p_sb = p_tiles[(h, c)]
            for half in range(2):
                sl = bass.ts(half, 512)
                pc = state[("pc", h, half)]
                nc.tensor.matmul(pc[:],
                                 (v_sb[:, c, hh * E:(hh + 1) * E]),
                                 (p_sb[:, sl]),
                                 start=(c == 0), stop=(c == NLK - 1))

        def emit_tadd(h, c):
            """DVE pairwise chunk-sums of P (fp16, 2x mode).

            Four partial sums T01/T23/T45/T67: each add lands right after
            its exp, so the head-boundary critical path is one add after
            the last exp.  Fewer partials would shave PE ones-matmuls but
            pushes ACT+DVE load too close to the PE's: one stall per head
            then drops the chip-wide clock ~20% (measured), which
            saturates ACT and locks the kernel in the slow state."""
            if c % 2 == 0:
                return
            t_sb = tpool.tile([128, LQ], F16, tag="T", name="t_sb")
            nc.vector.tensor_add(t_sb[:], p_tiles.pop((h, c - 1))[:],
                                 p_tiles.pop((h, c))[:])
            t_by_head.setdefault(h, []).append(t_sb)

        def emit_tmerge(h, i):
            """Merge two pair-sums so the PE's ones-matmul only streams 2
            partials instead of 4.  Merge 0 (mid-loop, off the critical
            path) goes to the otherwise-idle GpSimd (~2.1us); merge 1 sits
            on the head-boundary path right after the last exp, so it runs
            on the DVE (~0.7us) where the boundary PE filler covers it."""
            parts = t_by_head[h]
            m_sb = tmpool.tile([128, LQ], F16, tag="Tm", name="m_sb")
            eng = nc.gpsimd if i == 0 else nc.vector
            eng.tensor_add(m_sb[:], parts[2 * i][:], parts[2 * i + 1][:])
            state.setdefault(("tm", h), []).append(m_sb)

        def emit_norm(h):
            """rowsum (ones-matmuls over the merged partials) + recip + ctxn."""
            t_by_head.pop(h)
            t_parts = state.pop(("tm", h))
            recip_sb = rpool.tile([128, LQ], F32, tag="recip", name="recip_sb")
            ctxn_sb = cxpool.tile([E, LQ], F32R, tag="ctxn", name="ctxn_sb")
            ps_rs = [ps_t.tile([128, 512], F32, tag="t", name="ps_r")
                     for _ in range(2)]
            for i, t_sb in enumerate(t_parts):
                for half in range(2):
                    sl = bass.ts(half, 512)
                    nc.tensor.matmul(ps_rs[half][:], (ones_sb[:]), (t_sb[:, sl]),
                                     start=(i == 0), stop=(i == len(t_parts) - 1))
            for half in range(2):
                sl = bass.ts(half, 512)
                nc.vector.reciprocal_approx_fast(recip_sb[:, sl], ps_rs[half][:])
            for half in range(2):
                sl = bass.ts(half, 512)
                nc.vector.tensor_mul(ctxn_sb[:, sl],
                                     state.pop(("pc", h, half))[:],
                                     recip_sb[:, sl])
            state[("ctxn", h)] = ctxn_sb

        def emit_proj_half(h, half):
            """Half of the output projection for head h + DVE accumulate."""
            ctxn_sb = state[("ctxn", h)]
            wo_sb = state[("wo", h)]
            sl = bass.ts(half, 512)
            ps_p = ps_t.tile([E, 512], F32, tag="t", name="ps_p")
            nc.tensor.matmul(ps_p[:], (wo_sb[:]), (ctxn_sb[:, sl]),
                             start=True, stop=True)
            if h == 0:
                nc.vector.tensor_scalar_add(out_acc[:, sl], ps_p[:],
                                            bo2_sb[:, 0:1])
            else:
                nc.vector.tensor_add(out_acc[:, sl], out_acc[:, sl], ps_p[:])
            if half == 1:
                state.pop(("ctxn", h))
                state.pop(("wo", h))

        # ================= prologue =================
        dma_wk(0)
        emit_k_half(0, 0)
        nc.sync.dma_start(q_sb[:, bass.ts(0, 512)], qT[:, bass.ts(0, 512)])
        emit_k_half(0, 1)
        nc.sync.dma_start(q_sb[:, bass.ts(1, 512)], qT[:, bass.ts(1, 512)])
        dma_wv(0)
        nc.sync.dma_start(ones_sb[:], ones[:])
        nc.sync.dma_start(bo2_sb[:], bo2[:])
        emit_s(0, 0)
        for lk in range(NLK):
            emit_v_chunk(0, lk)
        emit_s(0, 1)
        dma_wk(1)
        dma_wv(1)
        emit_k_half(1, 0)
        emit_k_half(1, 1)

        # ================= head loop =================
        for h in range(H):
            hh = h % HPG
            g = h // HPG
            # boundary B(h) for h >= 1 (h == 0 handled by the prologue).
            # The K projection of head h+1 runs BEFORE the previous head's
            # normalization: it covers the exp7 -> T-add -> GpSimd-merge
            # latency so the ones-matmul input is ready when the PE gets
            # there.
            if h >= 1:
                if h + 1 < H:
                    dma_wk(h + 1)
                if hh == 0 and g + 1 < NG:
                    dma_wv(g + 1)
                wo_sb = wop.tile([E, E], F32R, tag="wo", name="wo_sb")
                nc.sync.dma_start(wo_sb[:], wo[(h - 1) * E:h * E, :])
                state[("wo", h - 1)] = wo_sb
                emit_s(h, 0)
                if h + 1 < H:
                    emit_k_half(h + 1, 0)
                emit_s(h, 1)
                if h + 1 < H:
                    emit_k_half(h + 1, 1)
                emit_norm(h - 1)          # ones-matmuls + recip + muls

            state[("pc", h, 0)] = ps_c.tile([E, 512], F32, tag="cA", name="ps_cA")
            state[("pc", h, 1)] = ps_c.tile([E, 512], F32, tag="cB", name="ps_cB")

            for c in range(NLK):
                # PE fillers between this chunk's dependencies; the
                # projection sits at c2/c3 (its ctxn comes from DVE muls
                # that trail the boundary ones-matmuls by ~1.5us)
                if c == 2 and h >= 1:
                    emit_proj_half(h - 1, 0)
                if c == 3 and h >= 1:
                    emit_proj_half(h - 1, 1)
                # V chunks at c6/c7: after S7's emission the exp chain is
                # S-gated, and AV6/AV7 need ~1.2us of PE cover each to not
                # stall on exp6/exp7
                if c in (6, 7) and g + 1 < NG:
                    emit_v_chunk(g + 1, 2 * hh + (c - 6))
                emit_av(h, c)
                emit_tadd(h, c)
                if c == 3:
                    emit_tmerge(h, 0)
                if c == 7:
                    emit_tmerge(h, 1)
                if c + 2 < NLK:
                    emit_s(h, c + 2)

        # ================= tail =================
        wo_sb = wop.tile([E, E], F32R, tag="wo", name="wo_sb")
        nc.sync.dma_start(wo_sb[:], wo[(H - 1) * E:H * E, :])
        state[("wo", H - 1)] = wo_sb
        emit_norm(H - 1)
        for half in range(2):
            emit_proj_half(H - 1, half)
            nc.sync.dma_start(outT[:, bass.ts(half, 512)],
                              out_acc[:, bass.ts(half, 512)])


def build_program():
    nc = bacc.Bacc("TRN2", target_bir_lowering=False, debug=False,
                   num_devices=N_CORES)
    qT = nc.dram_tensor("qT", [E, LQ], F32R, kind="ExternalInput").ap()
    sT = nc.dram_tensor("sT", [D, LK], F32R, kind="ExternalInput").ap()
    wk = nc.dram_tensor("wk", [H, D, E], F32R, kind="ExternalInput").ap()
    wv = nc.dram_tensor("wv", [D, H * E], F32R, kind="ExternalInput").ap()
    wo = nc.dram_tensor("wo", [H * E, E], F32R, kind="ExternalInput").ap()
    bo2 = nc.dram_tensor("bo2", [E, 1], F32, kind="ExternalInput").ap()
    ones = nc.dram_tensor("ones", [128, 128], F16, kind="ExternalInput").ap()
    outT = nc.dram_tensor("outT", [E, LQ], F32, kind="ExternalOutput").ap()

    with tile.TileContext(nc) as tc:
        _build_kernel(tc, qT, sT, wk, wv, wo, bo2, ones, outT)
    nc.compile()
    return nc


def _round_f32r(a):
    """Round fp32 -> fp32r (11-bit mantissa, low 12 bits zero), RN-even.

    The PE's fp32r datapath keeps sign+8exp+11mantissa; the BIR verifier
    requires fp32r matmul operands to be pre-rounded, and rounding on the
    host gives round-to-nearest instead of hardware truncation.
    """
    b = np.ascontiguousarray(a, dtype=np.float32).view(np.uint32)
    b = b + 0x7FF + ((b >> 12) & 1)
    b &= np.uint32(0xFFFFF000)
    return b.view(np.float32)


def make_in_maps(query, states, Wk, bk, Wv, bv, Wo, bo):
    """Shard the full inputs into per-core input maps (host-side prep)."""
    wv_packed = np.ascontiguousarray(
        np.transpose(Wv, (1, 0, 2)).reshape(D, H * E))
    # fold bv through the output projection: softmax rows sum to 1
    bo2 = bo.astype(np.float64).copy()
    for h in range(H):
        bo2 += bv[h].astype(np.float64) @ Wo[h * E:(h + 1) * E].astype(np.float64)
    bo2 = bo2.astype(np.float32).reshape(E, 1)
    wk_c = _round_f32r(Wk)
    wo_c = _round_f32r(Wo)
    wv_packed = _round_f32r(wv_packed)

    in_maps = []
    for b in range(B):
        in_maps.append({
            "qT": _round_f32r(query[b].T),
            "sT": _round_f32r(states[b].T),
            "wk": wk_c,
            "wv": wv_packed,
            "wo": wo_c,
            "bo2": bo2,
            "ones": np.ones((128, 128), dtype=np.float16),
        })
    return in_maps


_PROGRAM_CACHE = {}


def _get_program():
    if "nc" not in _PROGRAM_CACHE:
        _PROGRAM_CACHE["nc"] = build_program()
    return _PROGRAM_CACHE["nc"]


def kernel(query, states, Wk, bk, Wv, bv, Wo, bo, _trace=False, _tmpdir=None):
    args = [np.asarray(a, dtype=np.float32)
            for a in (query, states, Wk, bk, Wv, bv, Wo, bo)]
    nc = _get_program()
    in_maps = make_in_maps(*args)
    last_err = None
    for _attempt in range(2):  # one retry for transient device errors
        try:
            res = run_bass_kernel_spmd(nc, in_maps,
                                       core_ids=list(range(N_CORES)),
                                       trace=_trace, tmpdir=_tmpdir)
            break
        except Exception as e:  # noqa: BLE001
            last_err = e
    else:
        raise last_err
    out = np.stack([res.results[b]["outT"].T for b in range(B)])
    out = np.ascontiguousarray(out.astype(np.float32))
    if _trace:
        kernel.last_exec_time_ns = res.exec_time_ns
        kernel.last_results = res
    return out


if __name__ == "__main__":
    rng = np.random.default_rng(0)
    inputs = {
        "query": rng.standard_normal((B, LQ, E), dtype=np.float32),
        "states": rng.standard_normal((B, LK, D), dtype=np.float32),
        "Wk": rng.uniform(-0.04, 0.04, (H, D, E)).astype(np.float32),
        "bk": rng.uniform(-0.04, 0.04, (H, E)).astype(np.float32),
        "Wv": rng.uniform(-0.04, 0.04, (H, D, E)).astype(np.float32),
        "bv": rng.uniform(-0.04, 0.04, (H, E)).astype(np.float32),
        "Wo": rng.uniform(-0.015, 0.015, (H * E, E)).astype(np.float32),
        "bo": rng.uniform(-0.015, 0.015, (E,)).astype(np.float32),
    }
    out = kernel(**inputs)
    print(out.shape, out.dtype)
